# revision 2
# baseline (speedup 1.0000x reference)
"""Local 7x7-window per-channel attention (SASA-style) on 8 TRN2 NeuronCores.

v5: v4 + pipeline refinements.
  - X DMA split into 4 row-chunks matching the projection column chunks;
    weight/const DMAs issued on the ACT HWDGE queue so they land in parallel.
  - exp split per block into planes 0-3 / 4-6 with matching vmult quarters,
    so DVE's v-mult never waits on a full-block exp.
  - ACT-side radd planes (kw 5,6) emitted one block ahead of the exp that
    follows them in the ACT queue.
  - tail: S-folds and the 1/S chain (Ln, Exp — two table switches) overlap
    the O-side v-mults and folds.

Engine split per window plane:
  DVE : +r tensor_scalar (kw 0-4), q-mult, v-mult   (bf16 packed modes)
  ACT : +r identity-bias (kw 5,6), exp
  PE  : all 49-plane reductions (S = sum E, O = sum E*V) as identity-weight
        matmuls accumulating in PSUM fp32
K and V are stored twice (shifted one column) so window reads stay aligned.
Sharding: data-parallel, core c -> image c//2, row-half c%2, the half as two
14-row quarters stacked on partitions (p = quarter*64 + ch).
"""

import sys

if "/opt/trn_rl_repo" not in sys.path:
    sys.path.insert(0, "/opt/trn_rl_repo")

import numpy as np
import ml_dtypes

import concourse.bass as bass
import concourse.bacc as bacc
import concourse.tile as tile
from concourse import mybir
from concourse.bass_utils import run_bass_kernel_spmd

N_CORES = 8
KS = 7
PAD = 3
G = 8
DD = 8
C = 64
H = W = 56
B = 4
QR = 14
PR = QR + 2 * PAD  # 20
PW = W + 2 * PAD   # 62
NPIX = QR * W      # 784
HALF = NPIX // 2   # 392

F32 = mybir.dt.float32
BF16 = mybir.dt.bfloat16
ALU = mybir.AluOpType
ACTF = mybir.ActivationFunctionType


def build_nc():
    nc = bacc.Bacc("TRN2", target_bir_lowering=False, debug=False,
                   num_devices=N_CORES)
    x_ap = nc.dram_tensor("x", [128, PR, PW], F32, kind="ExternalInput").ap()
    # consts packed host-side: [wk | wq | wv | bk | bv | rt] = 128+128+128+1+1+49
    NCONST = 3 * 128 + 2 + KS * KS
    cst_ap = nc.dram_tensor("consts", [128, NCONST], F32,
                            kind="ExternalInput").ap()
    id_ap = nc.dram_tensor("ident", [128, 128], BF16, kind="ExternalInput").ap()
    out_ap = nc.dram_tensor("out", [128, QR, W], F32, kind="ExternalOutput").ap()

    with tile.TileContext(nc) as tc:
        with tc.tile_pool(name="const", bufs=1) as constp, \
             tc.tile_pool(name="planes", bufs=1) as planesp, \
             tc.tile_pool(name="blk", bufs=4) as blkp, \
             tc.tile_pool(name="evblk", bufs=2) as evp, \
             tc.tile_pool(name="small", bufs=1) as smallp, \
             tc.tile_pool(name="psum", bufs=3, space="PSUM") as psump, \
             tc.tile_pool(name="acc", bufs=1, space="PSUM") as accp:

            # consts in one DMA on the ACT HWDGE queue (parallel with X)
            CST = constp.tile([128, NCONST], F32)
            nc.scalar.dma_start(out=CST[:], in_=cst_ap[:])
            IDN = constp.tile([128, 128], BF16)
            nc.scalar.dma_start(out=IDN[:], in_=id_ap[:])
            RTOFF = 386

            # X in 2 row-chunks (10 rows = two 310-col projection chunks each)
            X = planesp.tile([128, PR, PW], F32)
            for j in range(2):
                nc.sync.dma_start(out=X[:, 10 * j:10 * (j + 1), :],
                                  in_=x_ap[:, 10 * j:10 * (j + 1), :])

            # preload the exp table set while DMAs run
            SCR = smallp.tile([128, 1], F32)
            nc.scalar.activation(out=SCR[:], in_=CST[:, 384:385], func=ACTF.Exp)

            K0 = planesp.tile([128, PR, PW], BF16)
            K1 = planesp.tile([128, PR, PW], BF16)
            V0 = planesp.tile([128, PR, PW], BF16)
            V1 = planesp.tile([128, PR, PW], BF16)
            Q = planesp.tile([128, QR, W], BF16)

            ncol = PR * PW  # 1240
            step = 310
            Xflat = X[:].rearrange("p h w -> p (h w)")

            def proj_kv(p0, p1, wmat, bias, k1_dve):
                f0 = p0[:].rearrange("p h w -> p (h w)")
                f1 = p1[:].rearrange("p h w -> p (h w)")
                for j in range(0, ncol, step):
                    ps = psump.tile([128, step], F32, tag="ps")
                    nc.tensor.matmul(ps[:], wmat, Xflat[:, j:j + step],
                                     start=True, stop=True)
                    nc.scalar.add(out=f0[:, j:j + step], in_=ps[:], add=bias)
                    if not k1_dve:
                        if j == 0:
                            nc.scalar.add(out=f1[:, 0:step - 1],
                                          in_=ps[:, 1:step], add=bias)
                        else:
                            nc.scalar.add(out=f1[:, j - 1:j + step - 1],
                                          in_=ps[:], add=bias)
                if k1_dve:
                    # shifted copy on DVE instead of a 2nd ACT pass
                    nc.vector.tensor_copy(f1[:, 0:ncol - 1], f0[:, 1:ncol])

            proj_kv(K0, K1, CST[:, 0:128], CST[:, 384:385], k1_dve=True)
            for j in range(2):
                ps = psump.tile([128, KS * W], F32, tag="ps")
                nc.tensor.matmul(
                    ps[:], CST[:, 128:256],
                    X[:, PAD + j * KS: PAD + (j + 1) * KS, PAD:PAD + W],
                    start=True, stop=True)
                nc.scalar.copy(
                    out=Q[:, j * KS:(j + 1) * KS, :].rearrange("p h w -> p (h w)"),
                    in_=ps[:])
            proj_kv(V0, V1, CST[:, 256:384], CST[:, 385:386], k1_dve=False)

            S0acc = accp.tile([128, HALF], F32, tag="S0")
            S1acc = accp.tile([128, HALF], F32, tag="S1")
            O0acc = accp.tile([128, HALF], F32, tag="O0")
            O1acc = accp.tile([128, HALF], F32, tag="O1")
            Sh = [S0acc, S1acc]
            Oh = [O0acc, O1acc]
            fold_n = {0: 0, 1: 0}  # matmuls issued per S/O accumulator pair

            qap = Q[:]
            qbcast = bass.AP(
                tensor=qap.tensor, offset=qap.offset,
                ap=[qap.ap[0], [0, KS], [W, QR], [1, W]])

            def win(img, kh, kw):
                par = kw & 1
                base = kw - par
                t = (K0, K1) if img == "k" else (V0, V1)
                return t[par][:, kh:kh + QR, base:base + W]

            def emit_radd_dve(kh, Ltile):
                for kw in range(6):
                    k = kh * KS + kw
                    nc.vector.tensor_scalar_add(
                        out=Ltile[:, kw], in0=win("k", kh, kw),
                        scalar1=CST[:, RTOFF + k:RTOFF + k + 1])

            def emit_radd_act(kh, Ltile, kws):
                for kw in kws:
                    k = kh * KS + kw
                    nc.scalar.activation(
                        out=Ltile[:, kw], in_=win("k", kh, kw),
                        func=ACTF.Identity, bias=CST[:, RTOFF + k:RTOFF + k + 1])

            def emit_qmult(Ltile):
                nc.vector.tensor_tensor(Ltile[:], Ltile[:], qbcast, ALU.mult)

            def emit_qmult_par(Ltile, par):
                lap = Ltile[:]
                lv = bass.AP(tensor=lap.tensor, offset=lap.offset + par * NPIX,
                             ap=[lap.ap[0], [2 * NPIX, 4 - par], [QR * W // QR, QR],
                                 [1, W]])
                qb = bass.AP(tensor=qap.tensor, offset=qap.offset,
                             ap=[qap.ap[0], [0, 4 - par], [W, QR], [1, W]])
                nc.vector.tensor_tensor(lv, lv, qb, ALU.mult)

            def emit_exp(Ltile, par):
                lap = Ltile[:]
                f = bass.AP(tensor=lap.tensor,
                            offset=lap.offset + par * NPIX,
                            ap=[lap.ap[0], [2 * NPIX, 4 - par], [1, NPIX]])
                nc.scalar.activation(out=f, in_=f, func=ACTF.Exp)

            def vplane_ap(vt, kh, base, nkw):
                vap = vt[:]
                return bass.AP(
                    tensor=vap.tensor, offset=vap.offset + kh * PW + base,
                    ap=[vap.ap[0], [2, nkw], [PW, QR], [1, W]])

            def emit_vmult(kh, Ltile, EVtile, par):
                n = 4 - par
                nc.vector.tensor_tensor(
                    EVtile[:, par:KS:2], Ltile[:, par:KS:2],
                    vplane_ap((V1 if par else V0), kh, 0, n), ALU.mult)

            def emit_fold(tileh, acc, accid, par):
                # accumulate parity-par planes of tileh into acc pair
                src = tileh[:].rearrange("p k h w -> p k (h w)")
                for kw in range(par, KS, 2):
                    for i in range(2):
                        rhs = bass.AP(
                            tensor=src.tensor,
                            offset=src.offset + kw * NPIX + i * HALF,
                            ap=[src.ap[0], [1, HALF]])
                        nc.tensor.matmul(
                            acc[i][:], IDN[:], rhs,
                            start=(fold_n[accid] == 0),
                            stop=(fold_n[accid] == KS * KS - 1),
                            skip_group_check=True)
                    fold_n[accid] += 1

            Lt = {}
            EVt = {}
            Lt[0] = blkp.tile([128, KS, QR, W], BF16, tag="L", name="L0")
            emit_radd_act(0, Lt[0], (6,))
            emit_radd_dve(0, Lt[0])

            for kh in range(KS):
                if kh == KS - 1:
                    emit_qmult_par(Lt[kh], 0)
                    emit_qmult_par(Lt[kh], 1)
                else:
                    emit_qmult(Lt[kh])
                if kh + 1 < KS:
                    Lt[kh + 1] = blkp.tile([128, KS, QR, W], BF16, tag="L",
                                           name=f"L{kh + 1}")
                    emit_radd_dve(kh + 1, Lt[kh + 1])
                emit_exp(Lt[kh], 0)
                emit_exp(Lt[kh], 1)
                if kh + 1 < KS:
                    emit_radd_act(kh + 1, Lt[kh + 1], (6,))
                EVt[kh] = evp.tile([128, KS, QR, W], BF16, tag="EV",
                                   name=f"EV{kh}")
                emit_vmult(kh, Lt[kh], EVt[kh], 0)
                emit_fold(Lt[kh], Sh, 0, 0)
                emit_vmult(kh, Lt[kh], EVt[kh], 1)
                emit_fold(Lt[kh], Sh, 0, 1)
                if kh == KS - 1:
                    # 1/S chain overlaps the O-side folds below
                    LNS = smallp.tile([128, NPIX], F32)
                    for i in range(2):
                        nc.scalar.activation(
                            out=LNS[:, i * HALF:(i + 1) * HALF],
                            in_=Sh[i][:], func=ACTF.Ln)
                    Rinv = smallp.tile([128, NPIX], F32)
                    nc.scalar.activation(out=Rinv[:], in_=LNS[:],
                                         func=ACTF.Exp, scale=-1.0)
                emit_fold(EVt[kh], Oh, 1, 0)
                emit_fold(EVt[kh], Oh, 1, 1)

            OUTC = smallp.tile([128, NPIX], F32)
            for i in range(2):
                nc.vector.tensor_mul(OUTC[:, i * HALF:(i + 1) * HALF],
                                     Oh[i][:], Rinv[:, i * HALF:(i + 1) * HALF])
                nc.sync.dma_start(
                    out=bass.AP(tensor=out_ap.tensor,
                                offset=out_ap.offset + i * HALF,
                                ap=[out_ap.ap[0], [1, HALF]]),
                    in_=OUTC[:, i * HALF:(i + 1) * HALF])

    nc.compile()
    return nc


def shard_inputs(x, wq, wk, bk, wv, bv, rel_x, rel_y):
    x_pad = np.zeros((B, C, H + 2 * PAD, W + 2 * PAD), dtype=np.float32)
    x_pad[:, :, PAD:PAD + H, PAD:PAD + W] = x

    def blockdiag(w):
        w64 = np.zeros((C, C), dtype=np.float32)
        for g in range(G):
            w64[g * DD:(g + 1) * DD, g * DD:(g + 1) * DD] = w[g].T
        w128 = np.zeros((128, 128), dtype=np.float32)
        w128[:64, :64] = w64
        w128[64:, 64:] = w64
        return w128

    wq128, wk128, wv128 = blockdiag(wq), blockdiag(wk), blockdiag(wv)
    bk128 = np.concatenate([bk, bk]).reshape(128, 1).astype(np.float32)
    bv128 = np.concatenate([bv, bv]).reshape(128, 1).astype(np.float32)

    rt64 = np.empty((C, KS, KS), dtype=np.float32)
    for g in range(G):
        for d in range(DD):
            if d < DD // 2:
                rt64[g * DD + d] = rel_x[d]
            else:
                rt64[g * DD + d] = rel_y[d - DD // 2]
    rt128 = np.concatenate([rt64, rt64]).reshape(128, KS * KS)
    rt128 = np.ascontiguousarray(rt128, dtype=np.float32)

    ident = np.eye(128, dtype=ml_dtypes.bfloat16)
    consts = np.concatenate(
        [wk128, wq128, wv128, bk128, bv128, rt128], axis=1).astype(np.float32)
    consts = np.ascontiguousarray(consts)

    in_maps = []
    for core in range(N_CORES):
        b, half = divmod(core, 2)
        r0 = half * 2 * QR
        xs = np.empty((128, PR, PW), dtype=np.float32)
        xs[:64] = x_pad[b, :, r0:r0 + PR, :]
        xs[64:] = x_pad[b, :, r0 + QR:r0 + QR + PR, :]
        in_maps.append({"x": xs, "consts": consts, "ident": ident})
    return in_maps


def unshard_output(results):
    out = np.empty((B, C, H, W), dtype=np.float32)
    for core in range(N_CORES):
        b, half = divmod(core, 2)
        r0 = half * 2 * QR
        r = results[core]["out"]
        out[b, :, r0:r0 + QR, :] = r[:64]
        out[b, :, r0 + QR:r0 + 2 * QR, :] = r[64:]
    return out


_NC_CACHE = {}


def get_nc():
    if "nc" not in _NC_CACHE:
        _NC_CACHE["nc"] = build_nc()
    return _NC_CACHE["nc"]


def kernel(**inputs):
    nc = get_nc()
    in_maps = shard_inputs(**inputs)
    res = run_bass_kernel_spmd(nc, in_maps, core_ids=list(range(N_CORES)))
    return unshard_output(res.results)


# revision 3
# speedup vs baseline: 1.2042x; 1.2042x over previous
"""Local 7x7-window per-channel attention (SASA-style) on 8 TRN2 NeuronCores.

Reference computation per (batch, channel, pixel):
  q = gconv1x1(x, wq); k = gconv1x1(pad(x), wk)+bk; v = likewise wv/bv
  logits[k] = q * (k_win[k] + r_c[k])  over the 49 window offsets,
  out = softmax(logits) @ v_win.

Engine split (the kernel is elementwise/transcendental bound, so all four
compute engines carry part of each 7-plane kh block):
  DVE : +r via tensor_scalar (kw 0..5, packed-bf16 2x), q-mult and the two
        parity v-mults (tensor_tensor 2x)
  ACT : +r for kw 6 (identity+bias), exp split by kw parity so the v-mults
        can start after half the block's exp
  PE  : BOTH 49-plane softmax reductions (S = sum E and O = sum E*V) as
        identity-weight matmuls accumulating into PSUM fp32 - 196 matmuls
        that replace all DVE tree-fold adds (and improve accuracy: fp32
        accumulation instead of bf16 tree sums)
  1/S via exp(-ln S); the Ln/Exp table switches overlap the O-side folds.
K and V images are stored twice (the second copy shifted one column) so
every (kh, kw) window view is 4B-aligned and DVE keeps its packed modes.
Startup: consts packed into one DMA on the ACT HWDGE queue in parallel
with X on the sync queue; K is projected first, then q, then V, so the
attention loop starts as soon as K and q exist.

Sharding: pure data-parallel, no collectives. Core c owns image c//2 and
output-row half c%2; the half is two 14-row quarters stacked on SBUF
partitions (p = quarter*64 + channel). Per-quarter padded input slab is
(64ch, 20rows, 62cols).
"""

import sys

if "/opt/trn_rl_repo" not in sys.path:
    sys.path.insert(0, "/opt/trn_rl_repo")

import numpy as np
import ml_dtypes

import concourse.bass as bass
import concourse.bacc as bacc
import concourse.tile as tile
from concourse import mybir
from concourse.bass_utils import run_bass_kernel_spmd

N_CORES = 8
KS = 7
PAD = 3
G = 8
DD = 8
C = 64
H = W = 56
B = 4
QR = 14
PR = QR + 2 * PAD  # 20
PW = W + 2 * PAD   # 62
NPIX = QR * W      # 784
HALF = NPIX // 2   # 392

F32 = mybir.dt.float32
BF16 = mybir.dt.bfloat16
ALU = mybir.AluOpType
ACTF = mybir.ActivationFunctionType


def build_nc():
    nc = bacc.Bacc("TRN2", target_bir_lowering=False, debug=False,
                   num_devices=N_CORES)
    x_ap = nc.dram_tensor("x", [128, PR, PW], F32, kind="ExternalInput").ap()
    # consts packed host-side: [wk | wq | wv | bk | bv | rt] = 128+128+128+1+1+49
    NCONST = 3 * 128 + 2 + KS * KS
    cst_ap = nc.dram_tensor("consts", [128, NCONST], F32,
                            kind="ExternalInput").ap()
    id_ap = nc.dram_tensor("ident", [128, 128], BF16, kind="ExternalInput").ap()
    out_ap = nc.dram_tensor("out", [128, QR, W], F32, kind="ExternalOutput").ap()

    with tile.TileContext(nc) as tc:
        with tc.tile_pool(name="const", bufs=1) as constp, \
             tc.tile_pool(name="planes", bufs=1) as planesp, \
             tc.tile_pool(name="blk", bufs=4) as blkp, \
             tc.tile_pool(name="evblk", bufs=2) as evp, \
             tc.tile_pool(name="small", bufs=1) as smallp, \
             tc.tile_pool(name="psum", bufs=3, space="PSUM") as psump, \
             tc.tile_pool(name="acc", bufs=1, space="PSUM") as accp:

            # consts in one DMA on the ACT HWDGE queue (parallel with X)
            CST = constp.tile([128, NCONST], F32)
            nc.scalar.dma_start(out=CST[:], in_=cst_ap[:])
            IDN = constp.tile([128, 128], BF16)
            nc.scalar.dma_start(out=IDN[:], in_=id_ap[:])
            RTOFF = 386

            # X in 2 row-chunks (10 rows = two 310-col projection chunks each)
            X = planesp.tile([128, PR, PW], F32)
            for j in range(2):
                nc.sync.dma_start(out=X[:, 10 * j:10 * (j + 1), :],
                                  in_=x_ap[:, 10 * j:10 * (j + 1), :])

            # preload the exp table set while DMAs run
            SCR = smallp.tile([128, 1], F32)
            nc.scalar.activation(out=SCR[:], in_=CST[:, 384:385], func=ACTF.Exp)

            K0 = planesp.tile([128, PR, PW], BF16)
            K1 = planesp.tile([128, PR, PW], BF16)
            V0 = planesp.tile([128, PR, PW], BF16)
            V1 = planesp.tile([128, PR, PW], BF16)
            Q = planesp.tile([128, QR, W], BF16)

            ncol = PR * PW  # 1240
            step = 310
            Xflat = X[:].rearrange("p h w -> p (h w)")

            def proj_kv(p0, p1, wmat, bias, k1_dve):
                f0 = p0[:].rearrange("p h w -> p (h w)")
                f1 = p1[:].rearrange("p h w -> p (h w)")
                for j in range(0, ncol, step):
                    ps = psump.tile([128, step], F32, tag="ps")
                    nc.tensor.matmul(ps[:], wmat, Xflat[:, j:j + step],
                                     start=True, stop=True)
                    nc.scalar.add(out=f0[:, j:j + step], in_=ps[:], add=bias)
                    if not k1_dve:
                        if j == 0:
                            nc.scalar.add(out=f1[:, 0:step - 1],
                                          in_=ps[:, 1:step], add=bias)
                        else:
                            nc.scalar.add(out=f1[:, j - 1:j + step - 1],
                                          in_=ps[:], add=bias)
                if k1_dve:
                    # shifted copy on DVE instead of a 2nd ACT pass
                    nc.vector.tensor_copy(f1[:, 0:ncol - 1], f0[:, 1:ncol])

            proj_kv(K0, K1, CST[:, 0:128], CST[:, 384:385], k1_dve=True)
            for j in range(2):
                ps = psump.tile([128, KS * W], F32, tag="ps")
                nc.tensor.matmul(
                    ps[:], CST[:, 128:256],
                    X[:, PAD + j * KS: PAD + (j + 1) * KS, PAD:PAD + W],
                    start=True, stop=True)
                nc.scalar.copy(
                    out=Q[:, j * KS:(j + 1) * KS, :].rearrange("p h w -> p (h w)"),
                    in_=ps[:])
            proj_kv(V0, V1, CST[:, 256:384], CST[:, 385:386], k1_dve=False)

            S0acc = accp.tile([128, HALF], F32, tag="S0")
            S1acc = accp.tile([128, HALF], F32, tag="S1")
            O0acc = accp.tile([128, HALF], F32, tag="O0")
            O1acc = accp.tile([128, HALF], F32, tag="O1")
            Sh = [S0acc, S1acc]
            Oh = [O0acc, O1acc]
            fold_n = {0: 0, 1: 0}  # matmuls issued per S/O accumulator pair

            qap = Q[:]
            qbcast = bass.AP(
                tensor=qap.tensor, offset=qap.offset,
                ap=[qap.ap[0], [0, KS], [W, QR], [1, W]])

            def win(img, kh, kw):
                par = kw & 1
                base = kw - par
                t = (K0, K1) if img == "k" else (V0, V1)
                return t[par][:, kh:kh + QR, base:base + W]

            def emit_radd_dve(kh, Ltile):
                for kw in range(6):
                    k = kh * KS + kw
                    nc.vector.tensor_scalar_add(
                        out=Ltile[:, kw], in0=win("k", kh, kw),
                        scalar1=CST[:, RTOFF + k:RTOFF + k + 1])

            def emit_radd_act(kh, Ltile, kws):
                for kw in kws:
                    k = kh * KS + kw
                    nc.scalar.activation(
                        out=Ltile[:, kw], in_=win("k", kh, kw),
                        func=ACTF.Identity, bias=CST[:, RTOFF + k:RTOFF + k + 1])

            def emit_qmult(Ltile):
                nc.vector.tensor_tensor(Ltile[:], Ltile[:], qbcast, ALU.mult)

            def emit_qmult_par(Ltile, par):
                lap = Ltile[:]
                lv = bass.AP(tensor=lap.tensor, offset=lap.offset + par * NPIX,
                             ap=[lap.ap[0], [2 * NPIX, 4 - par], [QR * W // QR, QR],
                                 [1, W]])
                qb = bass.AP(tensor=qap.tensor, offset=qap.offset,
                             ap=[qap.ap[0], [0, 4 - par], [W, QR], [1, W]])
                nc.vector.tensor_tensor(lv, lv, qb, ALU.mult)

            def emit_exp(Ltile, par):
                lap = Ltile[:]
                f = bass.AP(tensor=lap.tensor,
                            offset=lap.offset + par * NPIX,
                            ap=[lap.ap[0], [2 * NPIX, 4 - par], [1, NPIX]])
                nc.scalar.activation(out=f, in_=f, func=ACTF.Exp)

            def vplane_ap(vt, kh, base, nkw):
                vap = vt[:]
                return bass.AP(
                    tensor=vap.tensor, offset=vap.offset + kh * PW + base,
                    ap=[vap.ap[0], [2, nkw], [PW, QR], [1, W]])

            def emit_vmult(kh, Ltile, EVtile, par):
                n = 4 - par
                nc.vector.tensor_tensor(
                    EVtile[:, par:KS:2], Ltile[:, par:KS:2],
                    vplane_ap((V1 if par else V0), kh, 0, n), ALU.mult)

            def emit_fold(tileh, acc, accid, par):
                # accumulate parity-par planes of tileh into acc pair
                src = tileh[:].rearrange("p k h w -> p k (h w)")
                for kw in range(par, KS, 2):
                    for i in range(2):
                        rhs = bass.AP(
                            tensor=src.tensor,
                            offset=src.offset + kw * NPIX + i * HALF,
                            ap=[src.ap[0], [1, HALF]])
                        nc.tensor.matmul(
                            acc[i][:], IDN[:], rhs,
                            start=(fold_n[accid] == 0),
                            stop=(fold_n[accid] == KS * KS - 1),
                            skip_group_check=True)
                    fold_n[accid] += 1

            Lt = {}
            EVt = {}
            Lt[0] = blkp.tile([128, KS, QR, W], BF16, tag="L", name="L0")
            emit_radd_act(0, Lt[0], (6,))
            emit_radd_dve(0, Lt[0])

            for kh in range(KS):
                if kh == KS - 1:
                    emit_qmult_par(Lt[kh], 0)
                    emit_qmult_par(Lt[kh], 1)
                else:
                    emit_qmult(Lt[kh])
                if kh + 1 < KS:
                    Lt[kh + 1] = blkp.tile([128, KS, QR, W], BF16, tag="L",
                                           name=f"L{kh + 1}")
                    emit_radd_dve(kh + 1, Lt[kh + 1])
                emit_exp(Lt[kh], 0)
                emit_exp(Lt[kh], 1)
                if kh + 1 < KS:
                    emit_radd_act(kh + 1, Lt[kh + 1], (6,))
                EVt[kh] = evp.tile([128, KS, QR, W], BF16, tag="EV",
                                   name=f"EV{kh}")
                emit_vmult(kh, Lt[kh], EVt[kh], 0)
                emit_fold(Lt[kh], Sh, 0, 0)
                emit_vmult(kh, Lt[kh], EVt[kh], 1)
                emit_fold(Lt[kh], Sh, 0, 1)
                if kh == KS - 1:
                    # 1/S chain overlaps the O-side folds below
                    LNS = smallp.tile([128, NPIX], F32)
                    for i in range(2):
                        nc.scalar.activation(
                            out=LNS[:, i * HALF:(i + 1) * HALF],
                            in_=Sh[i][:], func=ACTF.Ln)
                    Rinv = smallp.tile([128, NPIX], F32)
                    nc.scalar.activation(out=Rinv[:], in_=LNS[:],
                                         func=ACTF.Exp, scale=-1.0)
                emit_fold(EVt[kh], Oh, 1, 0)
                emit_fold(EVt[kh], Oh, 1, 1)

            OUTC = smallp.tile([128, NPIX], F32)
            for i in range(2):
                nc.vector.tensor_mul(OUTC[:, i * HALF:(i + 1) * HALF],
                                     Oh[i][:], Rinv[:, i * HALF:(i + 1) * HALF])
                nc.sync.dma_start(
                    out=bass.AP(tensor=out_ap.tensor,
                                offset=out_ap.offset + i * HALF,
                                ap=[out_ap.ap[0], [1, HALF]]),
                    in_=OUTC[:, i * HALF:(i + 1) * HALF])

    nc.compile()
    return nc


def shard_inputs(x, wq, wk, bk, wv, bv, rel_x, rel_y):
    x_pad = np.zeros((B, C, H + 2 * PAD, W + 2 * PAD), dtype=np.float32)
    x_pad[:, :, PAD:PAD + H, PAD:PAD + W] = x

    def blockdiag(w):
        w64 = np.zeros((C, C), dtype=np.float32)
        for g in range(G):
            w64[g * DD:(g + 1) * DD, g * DD:(g + 1) * DD] = w[g].T
        w128 = np.zeros((128, 128), dtype=np.float32)
        w128[:64, :64] = w64
        w128[64:, 64:] = w64
        return w128

    wq128, wk128, wv128 = blockdiag(wq), blockdiag(wk), blockdiag(wv)
    bk128 = np.concatenate([bk, bk]).reshape(128, 1).astype(np.float32)
    bv128 = np.concatenate([bv, bv]).reshape(128, 1).astype(np.float32)

    rt64 = np.empty((C, KS, KS), dtype=np.float32)
    for g in range(G):
        for d in range(DD):
            if d < DD // 2:
                rt64[g * DD + d] = rel_x[d]
            else:
                rt64[g * DD + d] = rel_y[d - DD // 2]
    rt128 = np.concatenate([rt64, rt64]).reshape(128, KS * KS)
    rt128 = np.ascontiguousarray(rt128, dtype=np.float32)

    ident = np.eye(128, dtype=ml_dtypes.bfloat16)
    consts = np.concatenate(
        [wk128, wq128, wv128, bk128, bv128, rt128], axis=1).astype(np.float32)
    consts = np.ascontiguousarray(consts)

    in_maps = []
    for core in range(N_CORES):
        b, half = divmod(core, 2)
        r0 = half * 2 * QR
        xs = np.empty((128, PR, PW), dtype=np.float32)
        xs[:64] = x_pad[b, :, r0:r0 + PR, :]
        xs[64:] = x_pad[b, :, r0 + QR:r0 + QR + PR, :]
        in_maps.append({"x": xs, "consts": consts, "ident": ident})
    return in_maps


def unshard_output(results):
    out = np.empty((B, C, H, W), dtype=np.float32)
    for core in range(N_CORES):
        b, half = divmod(core, 2)
        r0 = half * 2 * QR
        r = results[core]["out"]
        out[b, :, r0:r0 + QR, :] = r[:64]
        out[b, :, r0 + QR:r0 + 2 * QR, :] = r[64:]
    return out


_NC_CACHE = {}


def get_nc():
    if "nc" not in _NC_CACHE:
        _NC_CACHE["nc"] = build_nc()
    return _NC_CACHE["nc"]


def kernel(**inputs):
    nc = get_nc()
    in_maps = shard_inputs(**inputs)
    res = run_bass_kernel_spmd(nc, in_maps, core_ids=list(range(N_CORES)))
    return unshard_output(res.results)
